# revision 9
# baseline (speedup 1.0000x reference)
"""Trainium2 Bass kernel for LocallyConnectedLinear.

Problem: out[b,h,w,o] = sum_k x_unf[b,h,w,k] * lc_params[h,w,k,o]
  x: [8,32,32,64] f32, lc_params: [30,30,576,64] f32 (k = c*9 + i*3 + j),
  out: [8,30,30,64] f32.

The weight tensor (132.7 MB, used once) dominates; the kernel is HBM-bound on
streaming it. Strategy:
  - Shard the 900 (h,w) output locations across the 8 cores (113/112 each,
    padded to 120 = 15 groups of 8). Each core streams only its weight slice.
  - Host-side im2col (a pure layout transform, same byte count as x itself)
    produces a K-major patch matrix xT [576, 960] per core (m = loc*8 + b).
  - Per group g of 8 locations: lhsT = xT[:, 64g:64g+64] (stationary,
    [K,64] = 8 locs x 8 batch), rhs = weights [K, 512] (8 locs x 64 outs),
    accumulated over 5 K-chunks into one PSUM bank [64, 512]. The useful
    outputs are the 8 diagonal [8b, 64o] blocks; the full [64,512] tile is
    copied out and the diagonal extracted on host (copying the whole tile is
    cheaper than 8 sparse on-chip copies).
"""

import json
import sys

for _p in ("/opt/trn_rl_repo/concourse", "/opt/trn_rl_repo"):
    if _p not in sys.path:
        sys.path.insert(0, _p)

import numpy as np

import concourse.bass as bass
import concourse.mybir as mybir
import concourse.tile as tile
from concourse.bass_utils import run_bass_kernel_spmd

N_CORES = 8
B = 8
H_OUT = W_OUT = 30
N_LOC = H_OUT * W_OUT  # 900
K = 576
O = 64
LOCS_PER_CORE = 120  # padded: 4 cores hold 113 real locs, 4 hold 112
GROUP = 8  # locations per matmul group (N = 8*64 = 512)
N_GROUPS = LOCS_PER_CORE // GROUP  # 15
K_CHUNKS = [(0, 128), (128, 128), (256, 128), (384, 128), (512, 64)]

# ---------------------------------------------------------------------------
# BIR post-pass: this walrus build accepts at most ONE sync wait per
# instruction (seen on Drain/TPB_CTRL and Matmult/S3_LW encodings), but Tile
# emits instructions carrying several waits (e.g. the kernel-tail drain
# gathers every dangling DMA semaphore). Hoist excess waits onto inserted
# single-wait EventSemaphore instructions immediately before the offender
# (same engine => same sequencer program order => identical semantics).
# ---------------------------------------------------------------------------
_WAIT_CAP = 1
_uid = [0]


def _hoist_inst(engine, wait, debug):
    _uid[0] += 1
    return {
        "debug": debug,
        "engine": engine,
        "ins": [],
        "name": f"hoistw-{_uid[0]}",
        "opcode": "EventSemaphore",
        "outs": [],
        "sync_info": {"on_update": [], "on_wait": [wait]},
    }


def _fix_bir_json(data: bytes) -> bytes:
    bir = json.loads(data)
    changed = False
    for fn in bir.get("functions", []):
        for blk in fn.get("blocks", []):
            out = []
            for ins in blk.get("instructions", []):
                si = ins.get("sync_info") or {}
                waits = si.get("on_wait") or []
                if len(waits) > _WAIT_CAP:
                    for w in waits[:-_WAIT_CAP]:
                        out.append(_hoist_inst(ins["engine"], w, ins.get("debug")))
                    si["on_wait"] = waits[-_WAIT_CAP:]
                    ins["sync_info"] = si
                    changed = True
                out.append(ins)
            blk["instructions"] = out
    return json.dumps(bir).encode() if changed else data


def _install_birfix(nc):
    orig = nc.to_json_bytes

    def patched(*a, **k):
        return _fix_bir_json(orig(*a, **k))

    nc.to_json_bytes = patched


# ---------------------------------------------------------------------------
# Bass kernel (identical NEFF on all 8 cores; per-core data differs)
# ---------------------------------------------------------------------------
GB = 5  # groups per weight-DMA block
N_BLOCKS = N_GROUPS // GB  # 3


def _build_nc(use_f32r=False):
    f32 = mybir.dt.float32
    wdt = mybir.dt.float32r if use_f32r else f32
    M = LOCS_PER_CORE * B  # 960
    BL = GB * GROUP  # 40 locs per block
    nc = bass.Bass()
    xT = nc.dram_tensor("xT", [K, M], wdt, kind="ExternalInput")
    # weights pre-transposed on host to [K, loc, O]: each SBUF partition (one
    # k-row) receives a fully contiguous (loc, o) run -> multi-KB descriptors
    wT = nc.dram_tensor("wT", [K, LOCS_PER_CORE, O], wdt, kind="ExternalInput")
    out = nc.dram_tensor("out", [N_GROUPS, GROUP * B, GROUP * O], f32,
                         kind="ExternalOutput")

    with tile.TileContext(nc) as tc:
        with (
            tc.tile_pool(name="xpool", bufs=1) as xpool,
            tc.tile_pool(name="wpool", bufs=8) as wpool,
            tc.tile_pool(name="psum", bufs=6, space="PSUM") as psum,
            tc.tile_pool(name="stage", bufs=4) as stage,
        ):
            xq = []
            for q0, kq in K_CHUNKS:
                xt = xpool.tile([kq, M], wdt, tag=f"x{q0}")
                nc.sync.dma_start(out=xt, in_=xT[q0 : q0 + kq, :])
                xq.append(xt)

            for blk in range(N_BLOCKS):
                wts = []
                for ci, (q0, kq) in enumerate(K_CHUNKS):
                    ring = nc.sync if ci % 2 == 0 else nc.scalar
                    wt = wpool.tile([kq, BL, O], wdt, tag="wt")
                    ring.dma_start(
                        out=wt,
                        in_=wT[q0 : q0 + kq, blk * BL : (blk + 1) * BL, :],
                    )
                    wts.append(wt)
                for gi in range(GB):
                    g = blk * GB + gi
                    lo, hi = g * GROUP, (g + 1) * GROUP
                    ps = psum.tile([GROUP * B, GROUP * O], f32)
                    for ci in range(len(K_CHUNKS)):
                        nc.tensor.matmul(
                            ps,
                            xq[ci][:, lo * B : hi * B],
                            wts[ci][:, gi * GROUP : (gi + 1) * GROUP, :],
                            start=(ci == 0),
                            stop=(ci == len(K_CHUNKS) - 1),
                        )
                    st = stage.tile([GROUP * B, GROUP * O], f32)
                    nc.vector.tensor_copy(st, ps)
                    engine = nc.scalar if gi % 2 == 0 else nc.sync
                    engine.dma_start(out=out[g], in_=st)

    _install_birfix(nc)
    return nc


# ---------------------------------------------------------------------------
# Host wrapper
# ---------------------------------------------------------------------------
def _core_splits():
    counts = [113] * 4 + [112] * 4  # sums to 900
    starts = np.cumsum([0] + counts[:-1]).tolist()
    return list(zip(starts, counts))


def kernel(x: np.ndarray, lc_params: np.ndarray) -> np.ndarray:
    x = np.ascontiguousarray(x, dtype=np.float32)
    lc = np.ascontiguousarray(lc_params, dtype=np.float32).reshape(N_LOC, K, O)

    # im2col, feature order (c, kh, kw) with c slowest: k = c*9 + i*3 + j
    sw = np.lib.stride_tricks.sliding_window_view(x, (3, 3), axis=(1, 2))
    x_unf = sw.reshape(B, N_LOC, K)  # [b, loc, k]

    in_maps = []
    for s, n in _core_splits():
        xTc = np.zeros((K, LOCS_PER_CORE * B), dtype=np.float32)
        # m = loc*8 + b  (b fastest)
        xTc[:, : n * B] = (
            x_unf[:, s : s + n, :].transpose(2, 1, 0).reshape(K, n * B)
        )
        wc = np.zeros((K, LOCS_PER_CORE, O), dtype=np.float32)
        wc[:, :n, :] = lc[s : s + n].transpose(1, 0, 2)
        in_maps.append({"xT": xTc, "wT": wc})

    nc = _build_nc()
    res = run_bass_kernel_spmd(nc, in_maps, core_ids=list(range(N_CORES)))

    out = np.empty((B, N_LOC, O), dtype=np.float32)
    idx = np.arange(GROUP)
    for core, (s, n) in enumerate(_core_splits()):
        r = res.results[core]["out"].reshape(N_GROUPS, GROUP, B, GROUP, O)
        # diagonal l == l2 -> [l, g, b, o] -> [b, g, l, o] -> [b, loc, o]
        d = r[:, idx, :, idx, :].transpose(2, 1, 0, 3).reshape(B, LOCS_PER_CORE, O)
        out[:, s : s + n, :] = d[:, :n, :]

    return out.reshape(B, H_OUT, W_OUT, O)


if __name__ == "__main__":
    xs = np.random.randn(B, 32, 32, 64).astype(np.float32)
    ws = (np.random.randn(H_OUT, W_OUT, K, O) * 0.02).astype(np.float32)
    r = kernel(xs, ws)
    print("kernel output shape:", r.shape)
